# revision 3
# baseline (speedup 1.0000x reference)
"""PadWithin2D (zero-insertion upsample, stride 2) on 8 Trainium2 NeuronCores.

Full input feats (16, 64, 128, 128) f32 -> output (16, 64, 256, 256) f32 with
out[:, :, ::2, ::2] = feats and zeros elsewhere.

Sharding: batch dim 16 -> 2 per core (trivially data-parallel, no
communication).  Per-core device kernel, per (b, c) image:
  - DMA image rows into SBUF (partition p = input row p),
  - DVE copies each row into the even columns of a zero-initialized row
    buffer (odd columns stay zero),
  - DMA the interleaved rows out to the even output rows.
Odd output rows are all-zero and are never written: run_bass_kernel_spmd
pre-zeros ExternalOutput buffers (donated zero buffers under the axon/PJRT
path -- see bass2jax.run_bass_via_pjrt), so the kernel writes only the
16 MiB of even rows instead of the full 32 MiB per core.
"""
import numpy as np
from contextlib import ExitStack

import concourse.bass as bass
import concourse.tile as tile
import concourse.mybir as mybir
from concourse import bacc
from concourse.bass_utils import run_bass_kernel_spmd

N_CORES = 8
B, C, H, W = 16, 64, 128, 128
S = 2
B_SH = B // N_CORES            # batches per core: 2
G = B_SH * C                   # images per core: 128
K = 16                         # images per pipeline iteration
N_IT = G // K                  # 8 iterations
N_BUF_OUT = 3                  # manual rotation for interleave tiles

_cached_nc = None


def _build():
    nc = bacc.Bacc("TRN2", target_bir_lowering=False, debug=False,
                   num_devices=N_CORES)
    fin = nc.declare_dram_parameter("feats", [B_SH, C, H, W],
                                    mybir.dt.float32, isOutput=False)
    fout = nc.declare_dram_parameter("out", [B_SH, C, H * S, W * S],
                                     mybir.dt.float32, isOutput=True)
    in_imgs = fin.rearrange("b c h w -> (b c) h w")     # [128, 128, 128]
    out_imgs = fout.rearrange("b c h w -> (b c) h w")   # [128, 256, 256]

    with tile.TileContext(nc) as tc:
        with ExitStack() as ctx:
            pin = ctx.enter_context(tc.tile_pool(name="pin", bufs=4))
            pout = ctx.enter_context(tc.tile_pool(name="pout", bufs=1))

            # Persistent interleave tiles, zeroed once; only even columns are
            # ever rewritten, so odd columns stay zero across iterations.
            out_tiles = []
            for bi in range(N_BUF_OUT):
                t = pout.tile([128, K * S * W], mybir.dt.float32,
                              tag=f"outbuf{bi}")
                half = K * S * W // 2
                nc.vector.memset(t[:, 0:half], 0.0)
                nc.gpsimd.memset(t[:, half:], 0.0)
                out_tiles.append(t)

            for it in range(N_IT):
                g0 = it * K
                tin = pin.tile([128, K * W], mybir.dt.float32, tag="inbuf")
                src = in_imgs[g0:g0 + K].rearrange("g h w -> h g w")
                nc.sync.dma_start(
                    tin[:].rearrange("p (k w) -> p k w", k=K), src)

                tout = out_tiles[it % N_BUF_OUT]
                nc.vector.tensor_copy(
                    tout[:].rearrange("p (k w) -> p k w", k=K)[:, :, 0::2],
                    tin[:].rearrange("p (k w) -> p k w", k=K))

                dst = out_imgs[g0:g0 + K, 0::2, :].rearrange("g h w -> h g w")
                nc.scalar.dma_start(
                    dst, tout[:].rearrange("p (k w) -> p k w", k=K))

    nc.compile()
    return nc


def _run(feats: np.ndarray, **spmd_kwargs):
    global _cached_nc
    if _cached_nc is None:
        _cached_nc = _build()
    feats = np.ascontiguousarray(feats, dtype=np.float32)
    in_maps = [{"feats": feats[i * B_SH:(i + 1) * B_SH]}
               for i in range(N_CORES)]
    res = run_bass_kernel_spmd(_cached_nc, in_maps, list(range(N_CORES)),
                               **spmd_kwargs)
    out = np.concatenate([res.results[i]["out"] for i in range(N_CORES)],
                         axis=0)
    return out, res


def kernel(feats: np.ndarray) -> np.ndarray:
    out, _ = _run(feats)
    return out
